# revision 6
# baseline (speedup 1.0000x reference)
"""DelayGNN stage kernel for 8 Trainium2 NeuronCores.

Strategy (graph/data parallel):
  - Nodes sharded across 8 cores (6400 padded nodes each); edge lists
    partitioned by destination core, sorted by destination block, grouped
    into 256-node destination blocks, padded to uniform chunk counts so one
    SPMD program serves all cores.
  - Per layer: bulk-gather x[src] rows (512B) from a replicated DRAM table
    with dma_gather (int16 indices; the node table is split in two halves to
    fit the int16 range), scatter-add into per-block accumulators with
    one-hot matmuls on the tensor engine (float32r), dense W matmuls in true
    fp32, row-layout epilogue (relu + residual + L2 normalize), then an
    AllGather of the new node features; the hop-2 aggregation (only needed
    by the next layer) overlaps the AllGather.
"""
import os
import sys
import numpy as np

for _p in ("/opt/trn_rl_repo", "/root/.axon_site/_ro/trn_rl_repo"):
    if os.path.isdir(_p) and _p not in sys.path:
        sys.path.append(_p)

P = 128
BLK = 256
NCORES = 8
HALF = 32768  # int16 index ceiling per gather table half


def _wrap_idx(flat):
    """[n] int -> dma_gather idx layout [128, n/16] (wrapped, replicated)."""
    n = len(flat)
    w = np.asarray(flat, np.int16).reshape(n // 16, 16).T  # [16, n/16]
    return np.ascontiguousarray(np.tile(w, (8, 1)))


def _prep_hop(src, dst, norm, n_per_core, nblk):
    """Partition edges by dst core, sort by (dst block, src half), pad each
    block to CA + CB chunks of 128. Returns (CA, CB, per-core tables)."""
    core = dst // n_per_core
    percore = []
    cntA = np.zeros(nblk, np.int64)
    cntB = np.zeros(nblk, np.int64)
    for k in range(NCORES):
        sel = core == k
        s, d, w = src[sel], dst[sel] - k * n_per_core, norm[sel]
        blk = d // BLK
        isB = (s >= HALF).astype(np.int64)
        order = np.lexsort((isB, blk))
        s, d, w, blk, isB = (a[order] for a in (s, d, w, blk, isB))
        grp = blk * 2 + isB
        cnt = np.bincount(grp, minlength=2 * nblk)
        starts = np.concatenate([[0], np.cumsum(cnt)[:-1]])
        rank = np.arange(len(s)) - starts[grp]
        percore.append((s, d, w, blk, isB, rank))
        cntA = np.maximum(cntA, cnt[0::2])
        cntB = np.maximum(cntB, cnt[1::2])
    # per-block chunk counts (max over cores -> SPMD-uniform program)
    CAb = np.maximum(1, -(-cntA // P)).astype(np.int64)
    CBb = (-(-cntB // P)).astype(np.int64)
    Cgb = CAb + CBb
    choff = np.concatenate([[0], np.cumsum(Cgb)])       # chunk offsets
    aoff = np.concatenate([[0], np.cumsum(CAb)])        # A-chunk offsets
    boff = np.concatenate([[0], np.cumsum(CBb)])        # B-chunk offsets
    J = int(Cgb.sum())
    out = []
    for k in range(NCORES):
        s, d, w, blk, isB, rank = percore[k]
        gix = np.zeros(J * P, np.int64)
        dp = np.zeros(J * P, np.float32)
        wp = np.zeros(J * P, np.float32)
        pos = choff[blk] * P + isB * (CAb[blk] * P) + rank
        gix[pos] = np.where(isB == 1, s - HALF, s)
        dp[pos] = (d % BLK).astype(np.float32)
        wp[pos] = w
        idxA = np.concatenate(
            [_wrap_idx(gix[choff[b] * P:(choff[b] + CAb[b]) * P])
             for b in range(nblk)], axis=1)
        idxB = (np.concatenate(
            [_wrap_idx(gix[(choff[b] + CAb[b]) * P:choff[b + 1] * P])
             for b in range(nblk) if CBb[b]], axis=1)
            if CBb.sum() else None)
        out.append((
            np.ascontiguousarray(dp.reshape(-1, P).T),
            np.ascontiguousarray(wp.reshape(-1, P).T),
            idxA, idxB,
        ))
    return tuple(CAb), tuple(CBb), out


def _edge_norm(src, dst, n):
    ones = np.ones(len(src), np.float32)
    deg_out = np.bincount(src, weights=ones, minlength=n).astype(np.float32)
    deg_in = np.bincount(dst, weights=ones, minlength=n).astype(np.float32)
    inv_out = np.where(deg_out > 0,
                       (1.0 / np.sqrt(np.maximum(deg_out, 1.0))), 0.0)
    inv_in = np.where(deg_in > 0,
                      (1.0 / np.sqrt(np.maximum(deg_in, 1.0))), 0.0)
    return (inv_out[src] * inv_in[dst]).astype(np.float32)


def _softmax(v):
    e = np.exp(v - v.max())
    return (e / e.sum()).astype(np.float32)


def _build(nblk, CA1, CB1, CA2, CB2, L, has_bias, msg_dt_name="float32r"):
    """Build the SPMD Bass program. nblk 256-dst blocks per core."""
    import concourse.bass as bass
    import concourse.tile as tile
    from concourse import bacc, mybir
    from concourse.library_config import mlp
    from contextlib import ExitStack

    F32 = mybir.dt.float32
    I16 = mybir.dt.int16
    MSG = getattr(mybir.dt, msg_dt_name)
    NP = nblk * BLK            # nodes per core
    NPAD = NP * NCORES
    HB = min(HALF, NPAD)       # rows in table half A
    NBN = NP // P              # 128-node blocks per core
    CAs, CBs = (np.asarray(CA1), np.asarray(CA2)), (np.asarray(CB1),
                                                     np.asarray(CB2))
    choffs = [np.concatenate([[0], np.cumsum(CAs[h] + CBs[h])])
              for h in range(2)]
    aoffs = [np.concatenate([[0], np.cumsum(CAs[h])]) for h in range(2)]
    boffs = [np.concatenate([[0], np.cumsum(CBs[h])]) for h in range(2)]
    Js = (int(choffs[0][-1]), int(choffs[1][-1]))

    nc = bacc.Bacc("TRN2", target_bir_lowering=False, debug=False,
                   num_devices=NCORES)

    x_full = nc.dram_tensor("x_full", [NPAD, P], F32, kind="ExternalInput")
    x_own = nc.dram_tensor("x_own", [NP, P], F32, kind="ExternalInput")
    w1d = nc.dram_tensor("w1s", [L, P, P], F32, kind="ExternalInput")
    w2d = nc.dram_tensor("w2s", [L, P, P], F32, kind="ExternalInput")
    biasd = nc.dram_tensor("biasb", [L, P, P], F32, kind="ExternalInput")
    iotad = nc.dram_tensor("iota", [P, BLK], F32, kind="ExternalInput")
    dstd = [nc.dram_tensor(f"dst{h}", [P, J], F32, kind="ExternalInput")
            for h, J in ((1, Js[0]), (2, Js[1]))]
    nrmd = [nc.dram_tensor(f"nrm{h}", [P, J], F32, kind="ExternalInput")
            for h, J in ((1, Js[0]), (2, Js[1]))]
    idxad = [nc.dram_tensor(f"idxa{h+1}", [P, int(aoffs[h][-1]) * 8], I16,
                            kind="ExternalInput") for h in range(2)]
    idxbd = [nc.dram_tensor(f"idxb{h+1}", [P, int(boffs[h][-1]) * 8], I16,
                            kind="ExternalInput") if CBs[h].sum() else None
             for h in range(2)]
    out_own = nc.dram_tensor("out_own", [NP, P], F32, kind="ExternalOutput")

    ag_in = [nc.dram_tensor(f"ag_in{t}", [NP, P], F32, kind="Internal")
             for t in range(L - 1)]
    ag_out = [nc.dram_tensor(f"ag_out{t}", [NPAD, P], F32, kind="Internal",
                             addr_space="Shared")
              for t in range(L - 1)]

    with tile.TileContext(nc) as tc, ExitStack() as ctx:
        sb = ctx.enter_context(tc.tile_pool(name="sb", bufs=1))
        gpool = ctx.enter_context(tc.tile_pool(name="g", bufs=3))
        ohpool = ctx.enter_context(tc.tile_pool(name="oh", bufs=4))
        accp = ctx.enter_context(
            tc.tile_pool(name="accp", bufs=2, space="PSUM"))
        densep = ctx.enter_context(
            tc.tile_pool(name="densep", bufs=2, space="PSUM"))
        misc = ctx.enter_context(tc.tile_pool(name="misc", bufs=2))

        # --- persistent tiles ---
        t_dst = [sb.tile([P, Js[0]], F32, tag="dst1", name="tdst1"),
                 sb.tile([P, Js[1]], F32, tag="dst2", name="tdst2")]
        t_nrm = [sb.tile([P, Js[0]], F32, tag="nrm1", name="tnrm1"),
                 sb.tile([P, Js[1]], F32, tag="nrm2", name="tnrm2")]
        t_ixa = [sb.tile([P, int(aoffs[h][-1]) * 8], I16, tag=f"ixa{h}",
                         name=f"ixa{h}") for h in range(2)]
        t_ixb = [sb.tile([P, int(boffs[h][-1]) * 8], I16, tag=f"ixb{h}",
                         name=f"ixb{h}") if CBs[h].sum() else None
                 for h in range(2)]
        t_iota = sb.tile([P, BLK], F32, tag="iota")
        t_w1 = sb.tile([P, L, P], F32, tag="w1")
        t_w2 = sb.tile([P, L, P], F32, tag="w2")
        t_bias = sb.tile([P, L, P], F32, tag="bias")
        x_rows = [sb.tile([P, NBN, P], F32, tag=f"xr{i}", name=f"xr{i}")
                  for i in range(2)]
        acc1 = sb.tile([P, nblk, BLK], F32, tag="acc1")
        acc2 = [sb.tile([P, nblk, BLK], F32, tag=f"acc2_{i}",
                        name=f"acc2_{i}") for i in range(2)]
        ssum = sb.tile([P, NBN], F32, tag="ssum")
        sinv = sb.tile([P, NBN], F32, tag="sinv")

        nc.gpsimd.load_library(mlp)
        for h in range(2):
            nc.sync.dma_start(t_dst[h][:], dstd[h][:])
            nc.sync.dma_start(t_nrm[h][:], nrmd[h][:])
            nc.sync.dma_start(t_ixa[h][:], idxad[h][:])
            if CBs[h].sum():
                nc.sync.dma_start(t_ixb[h][:], idxbd[h][:])
        nc.sync.dma_start(t_iota[:], iotad[:])
        nc.sync.dma_start(t_w1[:], w1d[:].rearrange("t i o -> i t o"))
        nc.sync.dma_start(t_w2[:], w2d[:].rearrange("t i o -> i t o"))
        nc.sync.dma_start(t_bias[:], biasd[:].rearrange("t i o -> i t o"))
        nc.sync.dma_start(x_rows[0][:],
                          x_own[:].rearrange("(a p) f -> p a f", p=P))

        MAXC = 8  # dma_gather descriptor-ring cap: <=1024 idxs per call

        def gather_pieces(tile_tag, tab_ap, idx_tile, ch0, C):
            """Gather C chunks (idx-table chunk offset ch0) in pieces of
            <=MAXC chunks. Returns [(first_chunk, piece_ap)]."""
            pieces = []
            for p0 in range(0, C, MAXC):
                pc = min(MAXC, C - p0)
                ni = pc * P
                xg = gpool.tile([P, pc, P], MSG, tag=f"{tile_tag}{p0}",
                                name=f"{tile_tag}{p0}")
                col0 = (ch0 + p0) * 8
                nc.gpsimd.dma_gather(
                    out_ap=xg[:], in_ap=tab_ap,
                    idxs_ap=idx_tile[:, col0:col0 + pc * 8],
                    num_idxs=ni, num_idxs_reg=ni, elem_size=P)
                pieces.append((p0, xg))
            return pieces

        def scatter_hop(h, acc_sb, xsrc):
            tab = xsrc.bitcast(MSG)
            for b in range(nblk):
                CA, CB = int(CAs[h][b]), int(CBs[h][b])
                Cg = CA + CB
                pa = gather_pieces("xga", tab[0:HB, :], t_ixa[h],
                                   int(aoffs[h][b]), CA)
                pb = (gather_pieces("xgb", tab[HB:NPAD, :], t_ixb[h],
                                    int(boffs[h][b]), CB)
                      if CB else [])

                def chunk_ap(c):
                    pieces, cc = (pa, c) if c < CA else (pb, c - CA)
                    for p0, xg in reversed(pieces):
                        if cc >= p0:
                            return xg[:, cc - p0, :]
                    raise AssertionError

                ps = accp.tile([P, BLK], F32, tag="psacc", space="PSUM")
                for c in range(Cg):
                    j = int(choffs[h][b]) + c
                    xsl = chunk_ap(c)
                    oh = ohpool.tile([P, BLK], MSG, tag="oh")
                    nc.vector.tensor_scalar(
                        out=oh[:], in0=t_iota[:],
                        scalar1=t_dst[h][:, j:j + 1],
                        scalar2=t_nrm[h][:, j:j + 1],
                        op0=mybir.AluOpType.is_equal,
                        op1=mybir.AluOpType.mult,
                    )
                    nc.tensor.matmul(out=ps[:], lhsT=xsl, rhs=oh[:],
                                     start=(c == 0), stop=(c == Cg - 1))
                nc.scalar.copy(acc_sb[:, b, :], ps[:])

        for t in range(L):
            xsrc = x_full[:] if t == 0 else ag_out[t - 1][:]
            xcur = x_rows[t % 2]
            xnew = x_rows[(t + 1) % 2]
            # hop1 aggregation (this layer)
            scatter_hop(0, acc1, xsrc)
            # dense + epilogue per 128-node block
            for nb in range(NBN):
                b, half = nb // 2, nb % 2
                ps = densep.tile([P, P], F32, tag="psd", space="PSUM")
                nc.tensor.matmul(
                    out=ps[:],
                    lhsT=acc1[:, b, half * P:(half + 1) * P],
                    rhs=t_w1[:, t, :], start=True, stop=(t == 0))
                if t > 0:
                    nc.tensor.matmul(
                        out=ps[:],
                        lhsT=acc2[(t + 1) % 2][:, b, half * P:(half + 1) * P],
                        rhs=t_w2[:, t, :], start=False, stop=True)
                u = misc.tile([P, P], F32, tag="u")
                if has_bias:
                    nc.vector.tensor_tensor(
                        out=u[:], in0=ps[:], in1=t_bias[:, t, :],
                        op=mybir.AluOpType.add)
                    nc.vector.tensor_scalar_max(u[:], u[:], 0.0)
                else:
                    nc.vector.tensor_scalar_max(u[:], ps[:], 0.0)
                nc.vector.tensor_tensor(
                    out=xnew[:, nb, :], in0=u[:], in1=xcur[:, nb, :],
                    op=mybir.AluOpType.add)
                sq = misc.tile([P, P], F32, tag="sq")
                nc.scalar.activation(
                    out=sq[:], in_=xnew[:, nb, :],
                    func=mybir.ActivationFunctionType.Square,
                    accum_out=ssum[:, nb:nb + 1])
            nc.scalar.sqrt(sinv[:], ssum[:])
            nc.vector.tensor_scalar_max(sinv[:], sinv[:], 1e-12)
            nc.vector.reciprocal(sinv[:], sinv[:])
            for nb in range(NBN):
                nc.scalar.activation(
                    out=xnew[:, nb, :], in_=xnew[:, nb, :],
                    func=mybir.ActivationFunctionType.Copy,
                    scale=sinv[:, nb:nb + 1])
            if t < L - 1:
                nc.sync.dma_start(
                    ag_in[t][:].rearrange("(a p) f -> p a f", p=P), xnew[:])
                nc.gpsimd.collective_compute(
                    "AllGather", mybir.AluOpType.bypass,
                    ins=[ag_in[t][:]], outs=[ag_out[t][:]],
                    replica_groups=[list(range(NCORES))],
                )
                # hop2 aggregation for next layer; overlaps the AllGather
                scatter_hop(1, acc2[t % 2], xsrc)
            else:
                nc.sync.dma_start(
                    out_own[:].rearrange("(a p) f -> p a f", p=P), xnew[:])
    nc.compile()
    return nc


def _prepare(x, W1, b1, W2, b2, alpha, src1, dst1, src2, dst2):
    N, D = x.shape
    L = W1.shape[0]
    assert D == P
    nblk = -(-N // (NCORES * BLK))
    NP = nblk * BLK
    NPAD = NP * NCORES

    norm1 = _edge_norm(src1, dst1, N)
    norm2 = _edge_norm(src2, dst2, N)
    CA1, CB1, tabs1 = _prep_hop(src1, dst1, norm1, NP, nblk)
    CA2, CB2, tabs2 = _prep_hop(src2, dst2, norm2, NP, nblk)

    a = np.zeros((L, 2), np.float32)
    a[0] = [1.0, 0.0]
    for t in range(1, L):
        a[t] = _softmax(alpha[t].astype(np.float32))
    w1s = (W1 * a[:, 0, None, None]).astype(np.float32)
    w2s = (W2 * a[:, 1, None, None]).astype(np.float32)
    bias = (a[:, 0, None] * b1 + a[:, 1, None] * b2).astype(np.float32)
    bias_b = np.broadcast_to(bias[:, None, :], (L, P, P)).copy()

    xpad = np.zeros((NPAD, P), np.float32)
    xpad[:N] = x
    iota = np.tile(np.arange(BLK, dtype=np.float32), (P, 1)).copy()

    in_maps = []
    for k in range(NCORES):
        m = dict(
            x_full=xpad, x_own=xpad[k * NP:(k + 1) * NP],
            w1s=w1s, w2s=w2s, biasb=bias_b, iota=iota,
            dst1=tabs1[k][0], nrm1=tabs1[k][1], idxa1=tabs1[k][2],
            dst2=tabs2[k][0], nrm2=tabs2[k][1], idxa2=tabs2[k][2],
        )
        if tabs1[k][3] is not None:
            m["idxb1"] = tabs1[k][3]
        if tabs2[k][3] is not None:
            m["idxb2"] = tabs2[k][3]
        in_maps.append(m)
    has_bias = bool(np.any(bias))
    return nblk, (CA1, CB1, CA2, CB2), L, N, NP, has_bias, in_maps


_CACHE = {}


def run(x, W1, b1, W2, b2, alpha, src1, dst1, src2, dst2,
        msg_dt_name="float32r", trace=False):
    from concourse import bass_utils
    nblk, Cs, L, N, NP, has_bias, in_maps = _prepare(
        x, W1, b1, W2, b2, alpha, src1, dst1, src2, dst2)
    key = (nblk,) + Cs + (L, has_bias, msg_dt_name)
    if key not in _CACHE:
        _CACHE[key] = _build(nblk, *Cs, L, has_bias, msg_dt_name)
    nc = _CACHE[key]
    res = bass_utils.run_bass_kernel_spmd(
        nc, in_maps, core_ids=list(range(NCORES)), trace=trace)
    out = np.concatenate([res.results[k]["out_own"] for k in range(NCORES)],
                         axis=0)[:N]
    return out, res


def kernel(x, W1, b1, W2, b2, alpha, src1, dst1, src2, dst2):
    out, _ = run(np.asarray(x, np.float32), np.asarray(W1, np.float32),
                 np.asarray(b1, np.float32), np.asarray(W2, np.float32),
                 np.asarray(b2, np.float32), np.asarray(alpha, np.float32),
                 np.asarray(src1, np.int32), np.asarray(dst1, np.int32),
                 np.asarray(src2, np.int32), np.asarray(dst2, np.int32))
    return out
